# revision 42
# baseline (speedup 1.0000x reference)
"""Trainium2 Bass kernel: causal multi-head attention, tensor-parallel over heads.

Problem: x:(2,2048,2048) f32, wq/wk/wv/wo:(2048,2048) f32 (torch Linear layout,
applied as x @ W.T).  out = MHA_causal(x) @ wo.T, 16 heads x 128 dim.

Sharding: 2 heads per core (8 cores).  Each core computes Q/K/V projections for
its 2 heads, causal attention, and a partial out-projection through its slice
of wo; the host sums the 8 partial outputs (the all-reduce equivalent).

Per-core layouts (all compute in fp16, accumulation in f32 PSUM):
  xT   (D, NTOK)   d-major      : streaming rhs for Q^T/K^T proj, stationary for V
  Q^T  (dh, tok)   per (b,h)    : moving operand of scores
  K^T  (dh, tok)   per (b,h)    : stationary operand of scores
  V    (tok, m)    token-major  : stationary operand of AV
  s_T  (key, q)    scores psum  : softmax denominator via DVE partition-tree
  y^T  (dh, tok)   normalized   : stationary operand of out-proj
"""

import sys

for _p in ("/opt/trn_rl_repo", "/root/.axon_site/_ro/trn_rl_repo"):
    if _p not in sys.path:
        sys.path.append(_p)

from contextlib import ExitStack

import ml_dtypes
import numpy as np

import concourse.bass as bass
import concourse.bacc as bacc
import concourse.mybir as mybir
import concourse.tile as tile
from concourse.bass_utils import run_bass_kernel_spmd

# compute dtype: fp16 (same TensorE speed as bf16, 4x DVE mode, and 3 more
# mantissa bits); PSUM accumulation is always f32
CDT = mybir.dt.float16
F32 = mybir.dt.float32
NPCDT = np.float16
# exp(x - 4*ln2) = exp(x)/16 keeps fp16 softmax denominators well inside range;
# the scale cancels exactly in the normalization
EXP_BIAS = -4.0 * float(np.log(2.0))

N_CORES = 8
B, S, D = 2, 2048, 2048
NH, DH = 16, 128
HPC = NH // N_CORES          # heads per core
ML = HPC * DH                # local head dims per core (256)
SCALE = 1.0 / float(np.sqrt(DH))


def build_nc(b=B, s=S, d=D):
    """Build the per-core Bass graph.  Parameterized so a scaled-down variant
    can run under CoreSim; the shipped kernel always uses the defaults."""
    ntok = b * s
    ndch = d // 128       # contraction chunks for projections
    nech = d // 512       # 512-wide out-proj e chunks
    ttch = s // 512       # 512-token chunks per batch
    nqj = s // 512        # 512-query chunks per (b, h)
    nblk = s // 128       # 128-token blocks per batch

    nc = bacc.Bacc("TRN2", target_bir_lowering=False, debug=False,
                   num_devices=N_CORES)
    # all inputs pre-tiled on the host (see _pretile / shard_inputs): every
    # DMA below is a long unit-stride run per partition
    xT_e = nc.dram_tensor("xT", [ntok // 512, 128, ndch * 512], CDT,
                          kind="ExternalInput").ap()
    wqT_e = nc.dram_tensor("wqT", [128, ndch * ML], CDT,
                           kind="ExternalInput").ap()
    wkT_e = nc.dram_tensor("wkT", [128, ndch * ML], CDT,
                           kind="ExternalInput").ap()
    wvT_e = nc.dram_tensor("wvT", [128, ndch * ML], CDT,
                           kind="ExternalInput").ap()
    woT_e = nc.dram_tensor("woT", [ML, d], CDT, kind="ExternalInput").ap()
    out_e = nc.dram_tensor("out", [ntok, d], CDT, kind="ExternalOutput").ap()

    with tile.TileContext(nc) as tc, ExitStack() as ctx:
        const = ctx.enter_context(tc.tile_pool(name="const", bufs=1))
        wpool = ctx.enter_context(tc.tile_pool(name="wpool", bufs=1))
        xpool = ctx.enter_context(tc.tile_pool(name="xpool", bufs=4))
        qkv = ctx.enter_context(tc.tile_pool(name="qkv", bufs=1))
        epool = ctx.enter_context(tc.tile_pool(name="epool", bufs=8))
        dpool = ctx.enter_context(tc.tile_pool(name="dpool", bufs=3))
        opool = ctx.enter_context(tc.tile_pool(name="opool", bufs=4))
        # explicit PSUM partitioning (8 banks total): 2 long-lived AV
        # accumulators (throttles attention to 2 blocks in flight), 4 for
        # score/projection groups (deep run-ahead so ACT never starves), 2
        # shared by denominator-broadcast and out-projection churn
        pso = ctx.enter_context(tc.tile_pool(name="pso", bufs=2, space="PSUM"))
        pss = ctx.enter_context(tc.tile_pool(name="pss", bufs=2, space="PSUM"))
        psj = ctx.enter_context(tc.tile_pool(name="psj", bufs=2, space="PSUM"))
        psm = ctx.enter_context(tc.tile_pool(name="psm", bufs=2, space="PSUM"))

        # --- constants -----------------------------------------------------
        # ones first: it gates the PE warm-up burst below
        ones_bf = const.tile([128, 128], CDT, tag="ones", name="ones")
        nc.gpsimd.memset(ones_bf[:], 1.0)

        # PE warm-up: dummy matmuls on the ones tile while the first DMAs are
        # still in flight, so the HAM clock gate is already open (2.4 GHz)
        # when real work arrives.  Sized to bridge until the first wq/x
        # pieces land (~2us) without delaying the first real matmul much.
        warm = psm.tile([128, 512], F32, tag="m", name="warm")
        for _ in range(36):
            nc.tensor.matmul(warm[:, 0:128], ones_bf[:], ones_bf[:],
                             start=True, stop=True, skip_group_check=True)
        wdump = const.tile([128, 128], F32, tag="wdump", name="wdump")
        nc.scalar.copy(wdump[:], warm[:, 0:128])

        # Diagonal causal mask: key row x attends query col y iff y - x >= 0.
        trimask = const.tile([128, 128], CDT, tag="trimask", name="trimask")
        nc.gpsimd.memset(trimask[:], 1.0)
        nc.gpsimd.affine_select(
            out=trimask[:], in_=trimask[:], compare_op=mybir.AluOpType.is_ge,
            fill=0.0, base=0, pattern=[[1, 128]], channel_multiplier=-1)
        ebias = const.tile([128, 1], F32, tag="ebias", name="ebias")
        nc.gpsimd.memset(ebias[:], EXP_BIAS)

        # --- weights: strict bandwidth-priority order on ONE queue.  The
        # first QK unit consumes (wq piece k, xt piece k) pairs in k order;
        # wk is needed one unit later, wv after the first V group, wo only
        # at out-projection.  A parallel queue would steal HBM bandwidth
        # from the critical first pieces, so everything goes on sync in
        # demand order. --------------------------------------------------
        wq_s = wpool.tile([128, ndch * ML], CDT, tag="wq", name="wq_s")
        wk_s = wpool.tile([128, ndch * ML], CDT, tag="wk", name="wk")
        xt0 = xpool.tile([128, ndch * 512], CDT, tag="xt", name="xt")
        w4 = ndch // 4
        # one DMA queue sustains only ~200 GB/s with ~2.5us issue->data
        # latency: split each piece group across the sync and scalar rings,
        # and make the FIRST piece a single k-chunk (wq k0 on sync || x k0
        # on scalar, ~0.19 MB) so the first matmul starts ~2us earlier
        qk_pieces = [(0, 1), (1, 4), (4, 8), (8, 12), (12, 16)]
        for a, bnd in qk_pieces:
            nc.sync.dma_start(
                wq_s[:, a * ML:bnd * ML],
                wqT_e[:, a * ML:bnd * ML])
            if bnd - a == 1:
                nc.scalar.dma_start(
                    xt0[:, a * 512:bnd * 512],
                    xT_e[0, :, a * 512:bnd * 512])
            else:
                nc.sync.dma_start(
                    xt0[:, a * 512:a * 512 + 512],
                    xT_e[0, :, a * 512:a * 512 + 512])
                nc.scalar.dma_start(
                    xt0[:, a * 512 + 512:bnd * 512],
                    xT_e[0, :, a * 512 + 512:bnd * 512])
        for piece in range(4):
            k0 = piece * w4
            nc.sync.dma_start(
                wk_s[:, k0 * ML:(k0 + w4) * ML],
                wkT_e[:, k0 * ML:(k0 + w4) * ML])
        wv_s = wpool.tile([128, ndch * ML], CDT, tag="wv", name="wv")
        nc.sync.dma_start(wv_s[:], wvT_e[:, :])
        wo_s = []
        for h in range(HPC):
            t = wpool.tile([128, d], CDT, tag=f"wo{h}", name=f"wo{h}")
            nc.sync.dma_start(t[:], woT_e[h * 128:(h + 1) * 128, :])
            wo_s.append(t)

        # --- persistent per-batch activations ------------------------------
        QT = [[qkv.tile([128, s], CDT, tag=f"qt{bb}{h}", name=f"qt{bb}{h}") for h in range(HPC)]
              for bb in range(b)]
        KT = [[qkv.tile([128, s], CDT, tag=f"kt{bb}{h}", name=f"kt{bb}{h}") for h in range(HPC)]
              for bb in range(b)]
        V = [qkv.tile([128, nblk * ML], CDT, tag=f"v{bb}", name=f"v{bb}") for bb in range(b)]
        YT = [[qkv.tile([128, s], CDT, tag=f"yt{bb}{h}", name=f"yt{bb}{h}") for h in range(HPC)]
              for bb in range(b)]

        xt_tiles = {(0, 0): xt0}

        def load_xt(bb):
            # emit the x-chunk DMAs early in the sync stream so they are
            # never head-of-line blocked behind output DMAs that wait on
            # attention results
            for tt in range(ttch):
                if (bb, tt) in xt_tiles:
                    continue
                c = (bb * s + tt * 512) // 512
                t = xpool.tile([128, ndch * 512], CDT, tag="xt", name="xt")
                nc.sync.dma_start(t[:], xT_e[c, :, :])
                xt_tiles[(bb, tt)] = t

        def qk_group(bb, tt, m2, wsb, dst):
            xt = xt_tiles[(bb, tt)]
            pst = psj.tile([128, 512], F32, tag="j", name="pst")
            for k in range(ndch):
                nc.tensor.matmul(
                    pst[:],
                    wsb[:, k * ML + m2 * 128: k * ML + m2 * 128 + 128],
                    xt[:, k * 512:(k + 1) * 512],
                    start=(k == 0), stop=(k == ndch - 1))
            # alternate the psum->sbuf cast between DVE and ACT so woven
            # projection groups don't pile onto the attention-loaded DVE
            if m2 % 2 == 0:
                nc.vector.tensor_copy(
                    dst[bb][m2][:, tt * 512:(tt + 1) * 512], pst[:])
            else:
                nc.scalar.copy(
                    dst[bb][m2][:, tt * 512:(tt + 1) * 512], pst[:])

        def qk_pair(bb, tt, wsb, dst, pool, ptag):
            # both m2 groups interleaved piece-by-piece: during startup the
            # weight/x pieces stream in at ~2.1us each while one group
            # consumes a piece in ~0.9us -- interleaving doubles the PE work
            # per piece so the DMA stream stays ahead.  The wk pair gets the
            # (still idle) score pool so it never waits on the wq pair's
            # psum casts.
            xt = xt_tiles[(bb, tt)]
            psts = [pool.tile([128, 512], F32, tag=ptag, name=f"pst{m2}")
                    for m2 in range(HPC)]
            for a, bnd in qk_pieces:
                for m2 in range(HPC):
                    for k in range(a, bnd):
                        nc.tensor.matmul(
                            psts[m2][:],
                            wsb[:, k * ML + m2 * 128: k * ML + m2 * 128 + 128],
                            xt[:, k * 512:(k + 1) * 512],
                            start=(k == 0), stop=(k == ndch - 1))
            for m2 in range(HPC):
                if m2 % 2 == 0:
                    nc.vector.tensor_copy(
                        dst[bb][m2][:, tt * 512:(tt + 1) * 512], psts[m2][:])
                else:
                    nc.scalar.copy(
                        dst[bb][m2][:, tt * 512:(tt + 1) * 512], psts[m2][:])

        def v_group(bb, tt, v4):
            xt = xt_tiles[(bb, tt)]
            pst = psj.tile([128, 512], F32, tag="j", name="pst")
            for k in range(ndch):
                nc.tensor.matmul(
                    pst[:, 0:ML],
                    xt[:, k * 512 + v4 * 128: k * 512 + v4 * 128 + 128],
                    wv_s[:, k * ML:(k + 1) * ML],
                    start=(k == 0), stop=(k == ndch - 1))
            blk = tt * 4 + v4
            nc.scalar.copy(V[bb][:, blk * ML:(blk + 1) * ML], pst[:, 0:ML])

        def proj_units(bb):
            # phase 1 as a list of independently emittable 16-matmul units so
            # the next batch's projection can be woven between attention
            # blocks of the current one (the per-engine schedule is
            # priority-ordered: stalls can only be filled by work that is
            # EMITTED inside the stall region)
            units = []
            for tt in range(ttch):
                # all wq units before wk units: matches the DMA priority
                # order at startup (the PE queue is in-order, so a unit
                # waiting on late data blocks every later-emitted unit).
                # The very first chunk uses the piece-interleaved pair form
                # to ride the incoming DMA stream without stalls.
                if bb == 0 and tt == 0:
                    units.append(lambda: qk_pair(0, 0, wq_s, QT, psj, "j"))
                    units.append(lambda: qk_pair(0, 0, wk_s, KT, pss, "s"))
                else:
                    for wsb, dst in ((wq_s, QT), (wk_s, KT)):
                        for m2 in range(HPC):
                            units.append(
                                lambda bb=bb, tt=tt, m2=m2, wsb=wsb, dst=dst:
                                qk_group(bb, tt, m2, wsb, dst))
                for v4 in range(4):
                    units.append(
                        lambda bb=bb, tt=tt, v4=v4: v_group(bb, tt, v4))
            return units

        def outproj_block(bb, n):
            # --- phase 3: partial out-projection for one 128-token block ---
            # fine-grained staging (one tile + DMA per 512-wide e chunk) so
            # the 32 MB output stream drains smoothly in the background
            t0 = n * 128
            # full-width (128, 2048) fp16 staging: one DMA per token block
            # with 4 KiB contiguous rows (2 KiB rows measured at only ~180
            # GB/s -- half rate); psum copies alternate ACT/DVE
            ost = opool.tile([128, d], CDT, tag="ost", name="ost")
            for ec in range(nech):
                # alternate psm/psj: 4 evacuation buffers keep the matmul
                # pairs (430ns/ec) ahead of the ~690ns psum->sbuf copies in
                # pure-outproj stretches (psj only carries the brief
                # denominator broadcast in the last batch's window)
                if ec % 2 == 0:
                    ps_p = psm.tile([128, 512], F32, tag="m", name="ps_p")
                else:
                    ps_p = psj.tile([128, 512], F32, tag="j", name="ps_p")
                for h in range(HPC):
                    nc.tensor.matmul(
                        ps_p[:],
                        YT[bb][h][:, t0:t0 + 128],
                        wo_s[h][:, ec * 512:(ec + 1) * 512],
                        start=(h == 0), stop=(h == HPC - 1))
                if ec % 2 == 0:
                    nc.scalar.copy(ost[:, ec * 512:(ec + 1) * 512], ps_p[:])
                else:
                    nc.vector.tensor_copy(
                        ost[:, ec * 512:(ec + 1) * 512], ps_p[:])
            nc.sync.dma_start(
                out_e[bb * s + t0: bb * s + t0 + 128, :], ost[:])

        def drain(fillers, k):
            for _ in range(min(k, len(fillers))):
                fillers.pop(0)()

        def attn_block(bb, qj, fillers=None, defer_to=None):
            # --- phase 2: causal attention for one 512-query block ---------
            for h in range(HPC):
                nki = 4 * qj + 4
                q0 = qj * 512
                ps_o = pso.tile([128, 512], F32, tag="o", name="ps_o")
                # single fp16 DVE accumulator chain: at ~370ns per add it
                # keeps up with et production (~520ns/tile), and the DVE is
                # in-order so a second chain would buy no latency -- while
                # one chain needs only ONE broadcast matmul per head-block
                acc = dpool.tile([128, 512], CDT, tag="acc", name="acc")
                for ki in range(nki):
                    # diagonal key chunks only see queries >= their own
                    # position: narrow to columns [c0:512)
                    g = ki - 4 * qj
                    c0 = 128 * g if g > 0 else 0
                    ps_s = pss.tile([128, 512], F32, tag="s", name="ps_s")
                    nc.tensor.matmul(
                        ps_s[:, c0:512],
                        KT[bb][h][:, ki * 128:(ki + 1) * 128],
                        QT[bb][h][:, q0 + c0:q0 + 512],
                        start=True, stop=True)
                    et = epool.tile([128, 512], CDT, tag="et", name="et")
                    nc.scalar.activation(
                        et[:, c0:512], ps_s[:, c0:512],
                        mybir.ActivationFunctionType.Exp, scale=SCALE,
                        bias=ebias[:, 0:1])
                    if g >= 0:
                        nc.vector.tensor_mul(
                            et[:, c0:c0 + 128], et[:, c0:c0 + 128],
                            trimask[:])
                    if ki == 0:
                        nc.vector.tensor_copy(acc[:, c0:512], et[:, c0:512])
                    else:
                        nc.vector.tensor_add(acc[:, c0:512], acc[:, c0:512],
                                             et[:, c0:512])
                    nc.tensor.matmul(
                        ps_o[:, c0:512],
                        V[bb][:, ki * ML + h * 128: ki * ML + h * 128 + 128],
                        et[:, c0:512],
                        start=(ki == 0), stop=(ki == nki - 1),
                        skip_group_check=True)
                    if fillers and ki % 3 == 2:
                        drain(fillers, 1)
                rbc = dpool.tile([128, 512], F32, tag="rbc", name="rbc")
                # one accumulating ones(128,128) matmul sums the 128
                # partitions of acc and broadcasts the denominator across
                # all 128 output partitions.  In the last batch the
                # projection PSUM pool (psj) is idle: use it there to keep
                # the broadcast off the outproj-churned psm pool.
                bpool = psj if bb + 1 == b else psm
                ps_r = bpool.tile([128, 512], F32, tag="j" if bb + 1 == b
                                  else "m", name="ps_r")
                nc.tensor.matmul(ps_r[:], ones_bf[:], acc[:],
                                 start=True, stop=True)
                nc.vector.reciprocal_approx_fast(out=rbc[:], in_=ps_r[:])
                nc.vector.tensor_mul(YT[bb][h][:, q0:q0 + 512],
                                     ps_o[:], rbc[:])
            for n in range(4 * qj, 4 * qj + 4):
                if defer_to is not None:
                    defer_to.append(lambda bb=bb, n=n: outproj_block(bb, n))
                else:
                    if fillers:
                        drain(fillers, 1)
                    outproj_block(bb, n)

        # per-batch stagger: the next batch's projection units are emitted
        # INSIDE the current batch's attention blocks (the per-engine
        # schedule is priority-ordered, so softmax-latency stalls can only
        # be filled by work emitted within them).  qj descending: longest
        # attention blocks first.
        deferred = []
        for bb in range(b):
            load_xt(bb)
            if bb == 0:
                for u in proj_units(0):
                    u()
            # fillers for this batch's attention window: the previous
            # batch's deferred out-projection plus the next batch's
            # projection units
            fillers = list(deferred)
            deferred = []
            if bb + 1 < b:
                load_xt(bb + 1)
                fillers += proj_units(bb + 1)
                defer_to = deferred   # push own out-proj into next window
            else:
                defer_to = None
            if bb + 1 < b:
                order = list(range(nqj - 1, -1, -1))
            else:
                # last batch: qj0 first (ready after the first projection
                # chunk, warms the softmax chain early), then longest-first
                order = [0] + list(range(nqj - 1, 0, -1))
            for qj in order:
                attn_block(bb, qj, fillers, defer_to)
            for u in fillers:
                u()

    nc.compile()
    return nc


_NC_CACHE = {}


def _get_nc():
    if "nc" not in _NC_CACHE:
        _NC_CACHE["nc"] = build_nc()
    return _NC_CACHE["nc"]


def _pretile(wT):
    """[d, m] -> [128, (d//128)*m] with per-partition-contiguous k-chunks:
    out[p, k*m + j] = wT[k*128 + p, j].  Makes every DMA a long unit-stride
    run per partition (fast descriptor gen + full HBM burst efficiency)."""
    d, m = wT.shape
    return np.ascontiguousarray(
        wT.reshape(d // 128, 128, m).transpose(1, 0, 2).reshape(128, -1))


def shard_inputs(x, wq, wk, wv, wo):
    """Host-side sharding: 2 heads (256 out dims) per core; fp16 cast."""
    ntok = x.shape[0] * x.shape[1]
    xf = np.asarray(x, dtype=np.float32).reshape(ntok, D)
    # x pre-tiled per 512-token chunk: xT3[c, p, k*512 + t] = x[c*512+t, k*128+p]
    nch = ntok // 512
    xT3 = np.ascontiguousarray(
        xf.reshape(nch, 512, D // 128, 128).transpose(0, 3, 2, 1)
        .reshape(nch, 128, -1)).astype(NPCDT)
    in_maps = []
    for c in range(N_CORES):
        sl = slice(c * ML, (c + 1) * ML)
        in_maps.append({
            "xT": xT3,
            "wqT": _pretile(np.asarray(wq)[sl].T).astype(NPCDT),
            "wkT": _pretile(np.asarray(wk)[sl].T).astype(NPCDT),
            "wvT": _pretile(np.asarray(wv)[sl].T).astype(NPCDT),
            "woT": np.ascontiguousarray(np.asarray(wo)[:, sl].T).astype(NPCDT),
        })
    return in_maps


def run(inputs, trace=False, trace_cores=None):
    nc = _get_nc()
    in_maps = shard_inputs(inputs["x"], inputs["wq"], inputs["wk"],
                           inputs["wv"], inputs["wo"])
    res = run_bass_kernel_spmd(nc, in_maps, core_ids=list(range(N_CORES)),
                               trace=trace, trace_cores=trace_cores)
    out = res.results[0]["out"].astype(np.float32)
    for c in range(1, N_CORES):
        out = out + res.results[c]["out"]
    return out.reshape(B, S, D), res


def kernel(**inputs) -> np.ndarray:
    out, _ = run(inputs, trace=False)
    return out



# revision 43
# speedup vs baseline: 1.1201x; 1.1201x over previous
"""Trainium2 Bass kernel: causal multi-head attention, tensor-parallel over heads.

Problem: x:(2,2048,2048) f32, wq/wk/wv/wo:(2048,2048) f32 (torch Linear layout,
applied as x @ W.T).  out = MHA_causal(x) @ wo.T, 16 heads x 128 dim.

Sharding: 2 heads per core (8 cores).  Each core computes Q/K/V projections for
its 2 heads, causal attention, and a partial out-projection through its slice
of wo; the host sums the 8 partial outputs (the all-reduce equivalent).

Per-core layouts (all compute in fp16, accumulation in f32 PSUM):
  xT   (D, NTOK)   d-major      : streaming rhs for Q^T/K^T proj, stationary for V
  Q^T  (dh, tok)   per (b,h)    : moving operand of scores
  K^T  (dh, tok)   per (b,h)    : stationary operand of scores
  V    (tok, m)    token-major  : stationary operand of AV
  s_T  (key, q)    scores psum  : softmax denominator via DVE partition-tree
  y^T  (dh, tok)   normalized   : stationary operand of out-proj
"""

import sys

for _p in ("/opt/trn_rl_repo", "/root/.axon_site/_ro/trn_rl_repo"):
    if _p not in sys.path:
        sys.path.append(_p)

from contextlib import ExitStack

import ml_dtypes
import numpy as np

import concourse.bass as bass
import concourse.bacc as bacc
import concourse.mybir as mybir
import concourse.tile as tile
from concourse.bass_utils import run_bass_kernel_spmd

# compute dtype: fp16 (same TensorE speed as bf16, 4x DVE mode, and 3 more
# mantissa bits); PSUM accumulation is always f32
CDT = mybir.dt.float16
F32 = mybir.dt.float32
NPCDT = np.float16
# exp(x - 4*ln2) = exp(x)/16 keeps fp16 softmax denominators well inside range;
# the scale cancels exactly in the normalization
EXP_BIAS = -4.0 * float(np.log(2.0))

N_CORES = 8
B, S, D = 2, 2048, 2048
NH, DH = 16, 128
HPC = NH // N_CORES          # heads per core
ML = HPC * DH                # local head dims per core (256)
SCALE = 1.0 / float(np.sqrt(DH))


def build_nc(b=B, s=S, d=D):
    """Build the per-core Bass graph.  Parameterized so a scaled-down variant
    can run under CoreSim; the shipped kernel always uses the defaults."""
    ntok = b * s
    ndch = d // 128       # contraction chunks for projections
    nech = d // 512       # 512-wide out-proj e chunks
    ttch = s // 512       # 512-token chunks per batch
    nqj = s // 512        # 512-query chunks per (b, h)
    nblk = s // 128       # 128-token blocks per batch

    nc = bacc.Bacc("TRN2", target_bir_lowering=False, debug=False,
                   num_devices=N_CORES)
    # all inputs pre-tiled on the host (see _pretile / shard_inputs): every
    # DMA below is a long unit-stride run per partition
    xT_e = nc.dram_tensor("xT", [ntok // 512, 128, ndch * 512], CDT,
                          kind="ExternalInput").ap()
    wqT_e = nc.dram_tensor("wqT", [128, ndch * ML], CDT,
                           kind="ExternalInput").ap()
    wkT_e = nc.dram_tensor("wkT", [128, ndch * ML], CDT,
                           kind="ExternalInput").ap()
    wvT_e = nc.dram_tensor("wvT", [128, ndch * ML], CDT,
                           kind="ExternalInput").ap()
    woT_e = nc.dram_tensor("woT", [ML, d], CDT, kind="ExternalInput").ap()
    out_e = nc.dram_tensor("out", [ntok, d], CDT, kind="ExternalOutput").ap()

    with tile.TileContext(nc) as tc, ExitStack() as ctx:
        const = ctx.enter_context(tc.tile_pool(name="const", bufs=1))
        wpool = ctx.enter_context(tc.tile_pool(name="wpool", bufs=1))
        xpool = ctx.enter_context(tc.tile_pool(name="xpool", bufs=4))
        qkv = ctx.enter_context(tc.tile_pool(name="qkv", bufs=1))
        epool = ctx.enter_context(tc.tile_pool(name="epool", bufs=8))
        dpool = ctx.enter_context(tc.tile_pool(name="dpool", bufs=3))
        opool = ctx.enter_context(tc.tile_pool(name="opool", bufs=4))
        # explicit PSUM partitioning (8 banks total): 2 long-lived AV
        # accumulators (throttles attention to 2 blocks in flight), 4 for
        # score/projection groups (deep run-ahead so ACT never starves), 2
        # shared by denominator-broadcast and out-projection churn
        pso = ctx.enter_context(tc.tile_pool(name="pso", bufs=2, space="PSUM"))
        pss = ctx.enter_context(tc.tile_pool(name="pss", bufs=2, space="PSUM"))
        psj = ctx.enter_context(tc.tile_pool(name="psj", bufs=2, space="PSUM"))
        psm = ctx.enter_context(tc.tile_pool(name="psm", bufs=2, space="PSUM"))

        # --- constants -----------------------------------------------------
        # ones first: it gates the PE warm-up burst below
        ones_bf = const.tile([128, 128], CDT, tag="ones", name="ones")
        nc.gpsimd.memset(ones_bf[:], 1.0)

        # PE warm-up: dummy matmuls on the ones tile while the first DMAs are
        # still in flight, so the HAM clock gate is already open (2.4 GHz)
        # when real work arrives.  Sized to bridge until the first wq/x
        # pieces land (~2us) without delaying the first real matmul much.
        warm = psm.tile([128, 512], F32, tag="m", name="warm")
        for _ in range(36):
            nc.tensor.matmul(warm[:, 0:128], ones_bf[:], ones_bf[:],
                             start=True, stop=True, skip_group_check=True)
        wdump = const.tile([128, 128], F32, tag="wdump", name="wdump")
        nc.scalar.copy(wdump[:], warm[:, 0:128])

        # Diagonal causal mask: key row x attends query col y iff y - x >= 0.
        trimask = const.tile([128, 128], CDT, tag="trimask", name="trimask")
        nc.gpsimd.memset(trimask[:], 1.0)
        nc.gpsimd.affine_select(
            out=trimask[:], in_=trimask[:], compare_op=mybir.AluOpType.is_ge,
            fill=0.0, base=0, pattern=[[1, 128]], channel_multiplier=-1)
        ebias = const.tile([128, 1], F32, tag="ebias", name="ebias")
        nc.gpsimd.memset(ebias[:], EXP_BIAS)

        # --- weights: strict bandwidth-priority order on ONE queue.  The
        # first QK unit consumes (wq piece k, xt piece k) pairs in k order;
        # wk is needed one unit later, wv after the first V group, wo only
        # at out-projection.  A parallel queue would steal HBM bandwidth
        # from the critical first pieces, so everything goes on sync in
        # demand order. --------------------------------------------------
        wq_s = wpool.tile([128, ndch * ML], CDT, tag="wq", name="wq_s")
        wk_s = wpool.tile([128, ndch * ML], CDT, tag="wk", name="wk")
        xt0 = xpool.tile([128, ndch * 512], CDT, tag="xt", name="xt")
        w4 = ndch // 4
        # one DMA queue sustains only ~200 GB/s with ~2.5us issue->data
        # latency: split each piece group across the sync and scalar rings,
        # and make the FIRST piece a single k-chunk (wq k0 on sync || x k0
        # on scalar, ~0.19 MB) so the first matmul starts ~2us earlier
        qk_pieces = [(0, 1), (1, 4), (4, 8), (8, 12), (12, 16)]
        for a, bnd in qk_pieces:
            nc.sync.dma_start(
                wq_s[:, a * ML:bnd * ML],
                wqT_e[:, a * ML:bnd * ML])
            if bnd - a == 1:
                nc.scalar.dma_start(
                    xt0[:, a * 512:bnd * 512],
                    xT_e[0, :, a * 512:bnd * 512])
            else:
                nc.sync.dma_start(
                    xt0[:, a * 512:a * 512 + 512],
                    xT_e[0, :, a * 512:a * 512 + 512])
                nc.scalar.dma_start(
                    xt0[:, a * 512 + 512:bnd * 512],
                    xT_e[0, :, a * 512 + 512:bnd * 512])
        # wk/wv/wo ride the scalar ring, which is empty once the x pieces
        # are out -- in parallel with the wq/x tail on sync, so the wk
        # units never wait (~2.5us earlier arrival than serialized on sync)
        for piece in range(4):
            k0 = piece * w4
            nc.scalar.dma_start(
                wk_s[:, k0 * ML:(k0 + w4) * ML],
                wkT_e[:, k0 * ML:(k0 + w4) * ML])
        wv_s = wpool.tile([128, ndch * ML], CDT, tag="wv", name="wv")
        nc.scalar.dma_start(wv_s[:], wvT_e[:, :])
        wo_s = []
        for h in range(HPC):
            t = wpool.tile([128, d], CDT, tag=f"wo{h}", name=f"wo{h}")
            nc.scalar.dma_start(t[:], woT_e[h * 128:(h + 1) * 128, :])
            wo_s.append(t)

        # --- persistent per-batch activations ------------------------------
        QT = [[qkv.tile([128, s], CDT, tag=f"qt{bb}{h}", name=f"qt{bb}{h}") for h in range(HPC)]
              for bb in range(b)]
        KT = [[qkv.tile([128, s], CDT, tag=f"kt{bb}{h}", name=f"kt{bb}{h}") for h in range(HPC)]
              for bb in range(b)]
        V = [qkv.tile([128, nblk * ML], CDT, tag=f"v{bb}", name=f"v{bb}") for bb in range(b)]
        YT = [[qkv.tile([128, s], CDT, tag=f"yt{bb}{h}", name=f"yt{bb}{h}") for h in range(HPC)]
              for bb in range(b)]

        xt_tiles = {(0, 0): xt0}

        def load_xt(bb):
            # emit the x-chunk DMAs early in the sync stream so they are
            # never head-of-line blocked behind output DMAs that wait on
            # attention results
            for tt in range(ttch):
                if (bb, tt) in xt_tiles:
                    continue
                c = (bb * s + tt * 512) // 512
                t = xpool.tile([128, ndch * 512], CDT, tag="xt", name="xt")
                nc.sync.dma_start(t[:], xT_e[c, :, :])
                xt_tiles[(bb, tt)] = t

        def qk_group(bb, tt, m2, wsb, dst):
            xt = xt_tiles[(bb, tt)]
            pst = psj.tile([128, 512], F32, tag="j", name="pst")
            for k in range(ndch):
                nc.tensor.matmul(
                    pst[:],
                    wsb[:, k * ML + m2 * 128: k * ML + m2 * 128 + 128],
                    xt[:, k * 512:(k + 1) * 512],
                    start=(k == 0), stop=(k == ndch - 1))
            # alternate the psum->sbuf cast between DVE and ACT so woven
            # projection groups don't pile onto the attention-loaded DVE
            if m2 % 2 == 0:
                nc.vector.tensor_copy(
                    dst[bb][m2][:, tt * 512:(tt + 1) * 512], pst[:])
            else:
                nc.scalar.copy(
                    dst[bb][m2][:, tt * 512:(tt + 1) * 512], pst[:])

        def qk_pair(bb, tt, wsb, dst, pool, ptag):
            # both m2 groups interleaved piece-by-piece: during startup the
            # weight/x pieces stream in at ~2.1us each while one group
            # consumes a piece in ~0.9us -- interleaving doubles the PE work
            # per piece so the DMA stream stays ahead.  The wk pair gets the
            # (still idle) score pool so it never waits on the wq pair's
            # psum casts.
            xt = xt_tiles[(bb, tt)]
            psts = [pool.tile([128, 512], F32, tag=ptag, name=f"pst{m2}")
                    for m2 in range(HPC)]
            for a, bnd in qk_pieces:
                for m2 in range(HPC):
                    for k in range(a, bnd):
                        nc.tensor.matmul(
                            psts[m2][:],
                            wsb[:, k * ML + m2 * 128: k * ML + m2 * 128 + 128],
                            xt[:, k * 512:(k + 1) * 512],
                            start=(k == 0), stop=(k == ndch - 1))
            for m2 in range(HPC):
                if m2 % 2 == 0:
                    nc.vector.tensor_copy(
                        dst[bb][m2][:, tt * 512:(tt + 1) * 512], psts[m2][:])
                else:
                    nc.scalar.copy(
                        dst[bb][m2][:, tt * 512:(tt + 1) * 512], psts[m2][:])

        def v_group(bb, tt, v4):
            xt = xt_tiles[(bb, tt)]
            pst = psj.tile([128, 512], F32, tag="j", name="pst")
            for k in range(ndch):
                nc.tensor.matmul(
                    pst[:, 0:ML],
                    xt[:, k * 512 + v4 * 128: k * 512 + v4 * 128 + 128],
                    wv_s[:, k * ML:(k + 1) * ML],
                    start=(k == 0), stop=(k == ndch - 1))
            blk = tt * 4 + v4
            nc.scalar.copy(V[bb][:, blk * ML:(blk + 1) * ML], pst[:, 0:ML])

        def proj_units(bb):
            # phase 1 as a list of independently emittable 16-matmul units so
            # the next batch's projection can be woven between attention
            # blocks of the current one (the per-engine schedule is
            # priority-ordered: stalls can only be filled by work that is
            # EMITTED inside the stall region)
            units = []
            for tt in range(ttch):
                # all wq units before wk units: matches the DMA priority
                # order at startup (the PE queue is in-order, so a unit
                # waiting on late data blocks every later-emitted unit).
                # The very first chunk uses the piece-interleaved pair form
                # to ride the incoming DMA stream without stalls.
                if bb == 0 and tt == 0:
                    units.append(lambda: qk_pair(0, 0, wq_s, QT, psj, "j"))
                    units.append(lambda: qk_pair(0, 0, wk_s, KT, pss, "s"))
                else:
                    for wsb, dst in ((wq_s, QT), (wk_s, KT)):
                        for m2 in range(HPC):
                            units.append(
                                lambda bb=bb, tt=tt, m2=m2, wsb=wsb, dst=dst:
                                qk_group(bb, tt, m2, wsb, dst))
                for v4 in range(4):
                    units.append(
                        lambda bb=bb, tt=tt, v4=v4: v_group(bb, tt, v4))
            return units

        def outproj_block(bb, n):
            # --- phase 3: partial out-projection for one 128-token block ---
            # fine-grained staging (one tile + DMA per 512-wide e chunk) so
            # the 32 MB output stream drains smoothly in the background
            t0 = n * 128
            # full-width (128, 2048) fp16 staging: one DMA per token block
            # with 4 KiB contiguous rows (2 KiB rows measured at only ~180
            # GB/s -- half rate); psum copies alternate ACT/DVE
            ost = opool.tile([128, d], CDT, tag="ost", name="ost")
            for ec in range(nech):
                # alternate psm/psj: 4 evacuation buffers keep the matmul
                # pairs (430ns/ec) ahead of the ~690ns psum->sbuf copies in
                # pure-outproj stretches (psj only carries the brief
                # denominator broadcast in the last batch's window)
                if ec % 2 == 0:
                    ps_p = psm.tile([128, 512], F32, tag="m", name="ps_p")
                else:
                    ps_p = psj.tile([128, 512], F32, tag="j", name="ps_p")
                for h in range(HPC):
                    nc.tensor.matmul(
                        ps_p[:],
                        YT[bb][h][:, t0:t0 + 128],
                        wo_s[h][:, ec * 512:(ec + 1) * 512],
                        start=(h == 0), stop=(h == HPC - 1))
                if ec % 2 == 0:
                    nc.scalar.copy(ost[:, ec * 512:(ec + 1) * 512], ps_p[:])
                else:
                    nc.vector.tensor_copy(
                        ost[:, ec * 512:(ec + 1) * 512], ps_p[:])
            nc.sync.dma_start(
                out_e[bb * s + t0: bb * s + t0 + 128, :], ost[:])

        def drain(fillers, k):
            for _ in range(min(k, len(fillers))):
                fillers.pop(0)()

        def attn_block(bb, qj, fillers=None, defer_to=None):
            # --- phase 2: causal attention for one 512-query block ---------
            for h in range(HPC):
                nki = 4 * qj + 4
                q0 = qj * 512
                ps_o = pso.tile([128, 512], F32, tag="o", name="ps_o")
                # single fp16 DVE accumulator chain: at ~370ns per add it
                # keeps up with et production (~520ns/tile), and the DVE is
                # in-order so a second chain would buy no latency -- while
                # one chain needs only ONE broadcast matmul per head-block
                acc = dpool.tile([128, 512], CDT, tag="acc", name="acc")
                for ki in range(nki):
                    # diagonal key chunks only see queries >= their own
                    # position: narrow to columns [c0:512)
                    g = ki - 4 * qj
                    c0 = 128 * g if g > 0 else 0
                    ps_s = pss.tile([128, 512], F32, tag="s", name="ps_s")
                    nc.tensor.matmul(
                        ps_s[:, c0:512],
                        KT[bb][h][:, ki * 128:(ki + 1) * 128],
                        QT[bb][h][:, q0 + c0:q0 + 512],
                        start=True, stop=True)
                    et = epool.tile([128, 512], CDT, tag="et", name="et")
                    nc.scalar.activation(
                        et[:, c0:512], ps_s[:, c0:512],
                        mybir.ActivationFunctionType.Exp, scale=SCALE,
                        bias=ebias[:, 0:1])
                    if g >= 0:
                        nc.vector.tensor_mul(
                            et[:, c0:c0 + 128], et[:, c0:c0 + 128],
                            trimask[:])
                    if ki == 0:
                        nc.vector.tensor_copy(acc[:, c0:512], et[:, c0:512])
                    else:
                        nc.vector.tensor_add(acc[:, c0:512], acc[:, c0:512],
                                             et[:, c0:512])
                    nc.tensor.matmul(
                        ps_o[:, c0:512],
                        V[bb][:, ki * ML + h * 128: ki * ML + h * 128 + 128],
                        et[:, c0:512],
                        start=(ki == 0), stop=(ki == nki - 1),
                        skip_group_check=True)
                    if fillers and ki % 3 == 2:
                        drain(fillers, 1)
                rbc = dpool.tile([128, 512], F32, tag="rbc", name="rbc")
                # one accumulating ones(128,128) matmul sums the 128
                # partitions of acc and broadcasts the denominator across
                # all 128 output partitions.  In the last batch the
                # projection PSUM pool (psj) is idle: use it there to keep
                # the broadcast off the outproj-churned psm pool.
                bpool = psj if bb + 1 == b else psm
                ps_r = bpool.tile([128, 512], F32, tag="j" if bb + 1 == b
                                  else "m", name="ps_r")
                nc.tensor.matmul(ps_r[:], ones_bf[:], acc[:],
                                 start=True, stop=True)
                nc.vector.reciprocal_approx_fast(out=rbc[:], in_=ps_r[:])
                nc.vector.tensor_mul(YT[bb][h][:, q0:q0 + 512],
                                     ps_o[:], rbc[:])
            for n in range(4 * qj, 4 * qj + 4):
                if defer_to is not None:
                    defer_to.append(lambda bb=bb, n=n: outproj_block(bb, n))
                else:
                    if fillers:
                        drain(fillers, 1)
                    outproj_block(bb, n)

        # per-batch stagger: the next batch's projection units are emitted
        # INSIDE the current batch's attention blocks (the per-engine
        # schedule is priority-ordered, so softmax-latency stalls can only
        # be filled by work emitted within them).  qj descending: longest
        # attention blocks first.
        deferred = []
        for bb in range(b):
            load_xt(bb)
            if bb == 0:
                for u in proj_units(0):
                    u()
            # fillers for this batch's attention window: the previous
            # batch's deferred out-projection plus the next batch's
            # projection units
            fillers = list(deferred)
            deferred = []
            if bb + 1 < b:
                load_xt(bb + 1)
                fillers += proj_units(bb + 1)
                defer_to = deferred   # push own out-proj into next window
            else:
                defer_to = None
            if bb + 1 < b:
                order = list(range(nqj - 1, -1, -1))
            else:
                # last batch: qj0 first (ready after the first projection
                # chunk, warms the softmax chain early), then longest-first
                order = [0] + list(range(nqj - 1, 0, -1))
            for qj in order:
                attn_block(bb, qj, fillers, defer_to)
            for u in fillers:
                u()

    nc.compile()
    return nc


_NC_CACHE = {}


def _get_nc():
    if "nc" not in _NC_CACHE:
        _NC_CACHE["nc"] = build_nc()
    return _NC_CACHE["nc"]


def _pretile(wT):
    """[d, m] -> [128, (d//128)*m] with per-partition-contiguous k-chunks:
    out[p, k*m + j] = wT[k*128 + p, j].  Makes every DMA a long unit-stride
    run per partition (fast descriptor gen + full HBM burst efficiency)."""
    d, m = wT.shape
    return np.ascontiguousarray(
        wT.reshape(d // 128, 128, m).transpose(1, 0, 2).reshape(128, -1))


def shard_inputs(x, wq, wk, wv, wo):
    """Host-side sharding: 2 heads (256 out dims) per core; fp16 cast."""
    ntok = x.shape[0] * x.shape[1]
    xf = np.asarray(x, dtype=np.float32).reshape(ntok, D)
    # x pre-tiled per 512-token chunk: xT3[c, p, k*512 + t] = x[c*512+t, k*128+p]
    nch = ntok // 512
    xT3 = np.ascontiguousarray(
        xf.reshape(nch, 512, D // 128, 128).transpose(0, 3, 2, 1)
        .reshape(nch, 128, -1)).astype(NPCDT)
    in_maps = []
    for c in range(N_CORES):
        sl = slice(c * ML, (c + 1) * ML)
        in_maps.append({
            "xT": xT3,
            "wqT": _pretile(np.asarray(wq)[sl].T).astype(NPCDT),
            "wkT": _pretile(np.asarray(wk)[sl].T).astype(NPCDT),
            "wvT": _pretile(np.asarray(wv)[sl].T).astype(NPCDT),
            "woT": np.ascontiguousarray(np.asarray(wo)[:, sl].T).astype(NPCDT),
        })
    return in_maps


def run(inputs, trace=False, trace_cores=None):
    nc = _get_nc()
    in_maps = shard_inputs(inputs["x"], inputs["wq"], inputs["wk"],
                           inputs["wv"], inputs["wo"])
    res = run_bass_kernel_spmd(nc, in_maps, core_ids=list(range(N_CORES)),
                               trace=trace, trace_cores=trace_cores)
    out = res.results[0]["out"].astype(np.float32)
    for c in range(1, N_CORES):
        out = out + res.results[c]["out"]
    return out.reshape(B, S, D), res


def kernel(**inputs) -> np.ndarray:
    out, _ = run(inputs, trace=False)
    return out



# revision 44
# speedup vs baseline: 1.1810x; 1.0544x over previous
"""Trainium2 Bass kernel: causal multi-head attention, tensor-parallel over heads.

Problem: x:(2,2048,2048) f32, wq/wk/wv/wo:(2048,2048) f32 (torch Linear layout,
applied as x @ W.T).  out = MHA_causal(x) @ wo.T, 16 heads x 128 dim.

Sharding: 2 heads per core (8 cores).  Each core computes Q/K/V projections for
its 2 heads, causal attention, and a partial out-projection through its slice
of wo; the host sums the 8 partial outputs (the all-reduce equivalent).

Per-core layouts (all compute in fp16, accumulation in f32 PSUM):
  xT   (D, NTOK)   d-major      : streaming rhs for Q^T/K^T proj, stationary for V
  Q^T  (dh, tok)   per (b,h)    : moving operand of scores
  K^T  (dh, tok)   per (b,h)    : stationary operand of scores
  V    (tok, m)    token-major  : stationary operand of AV
  s_T  (key, q)    scores psum  : softmax denominator via DVE partition-tree
  y^T  (dh, tok)   normalized   : stationary operand of out-proj
"""

import sys

for _p in ("/opt/trn_rl_repo", "/root/.axon_site/_ro/trn_rl_repo"):
    if _p not in sys.path:
        sys.path.append(_p)

from contextlib import ExitStack

import ml_dtypes
import numpy as np

import concourse.bass as bass
import concourse.bacc as bacc
import concourse.mybir as mybir
import concourse.tile as tile
from concourse.bass_utils import run_bass_kernel_spmd

# compute dtype: fp16 (same TensorE speed as bf16, 4x DVE mode, and 3 more
# mantissa bits); PSUM accumulation is always f32
CDT = mybir.dt.float16
F32 = mybir.dt.float32
NPCDT = np.float16
# exp(x - 4*ln2) = exp(x)/16 keeps fp16 softmax denominators well inside range;
# the scale cancels exactly in the normalization
EXP_BIAS = -4.0 * float(np.log(2.0))

N_CORES = 8
B, S, D = 2, 2048, 2048
NH, DH = 16, 128
HPC = NH // N_CORES          # heads per core
ML = HPC * DH                # local head dims per core (256)
SCALE = 1.0 / float(np.sqrt(DH))


def build_nc(b=B, s=S, d=D):
    """Build the per-core Bass graph.  Parameterized so a scaled-down variant
    can run under CoreSim; the shipped kernel always uses the defaults."""
    ntok = b * s
    ndch = d // 128       # contraction chunks for projections
    nech = d // 512       # 512-wide out-proj e chunks
    ttch = s // 512       # 512-token chunks per batch
    nqj = s // 512        # 512-query chunks per (b, h)
    nblk = s // 128       # 128-token blocks per batch

    nc = bacc.Bacc("TRN2", target_bir_lowering=False, debug=False,
                   num_devices=N_CORES)
    # all inputs pre-tiled on the host (see _pretile / shard_inputs): every
    # DMA below is a long unit-stride run per partition
    xT_e = nc.dram_tensor("xT", [ntok // 512, 128, ndch * 512], CDT,
                          kind="ExternalInput").ap()
    wqT_e = nc.dram_tensor("wqT", [128, ndch * ML], CDT,
                           kind="ExternalInput").ap()
    wkT_e = nc.dram_tensor("wkT", [128, ndch * ML], CDT,
                           kind="ExternalInput").ap()
    wvT_e = nc.dram_tensor("wvT", [128, ndch * ML], CDT,
                           kind="ExternalInput").ap()
    woT_e = nc.dram_tensor("woT", [ML, d], CDT, kind="ExternalInput").ap()
    out_e = nc.dram_tensor("out", [ntok, d], CDT, kind="ExternalOutput").ap()

    with tile.TileContext(nc) as tc, ExitStack() as ctx:
        const = ctx.enter_context(tc.tile_pool(name="const", bufs=1))
        wpool = ctx.enter_context(tc.tile_pool(name="wpool", bufs=1))
        xpool = ctx.enter_context(tc.tile_pool(name="xpool", bufs=4))
        qkv = ctx.enter_context(tc.tile_pool(name="qkv", bufs=1))
        epool = ctx.enter_context(tc.tile_pool(name="epool", bufs=8))
        dpool = ctx.enter_context(tc.tile_pool(name="dpool", bufs=3))
        opool = ctx.enter_context(tc.tile_pool(name="opool", bufs=4))
        # explicit PSUM partitioning (8 banks total): 2 long-lived AV
        # accumulators (throttles attention to 2 blocks in flight), 4 for
        # score/projection groups (deep run-ahead so ACT never starves), 2
        # shared by denominator-broadcast and out-projection churn
        pso = ctx.enter_context(tc.tile_pool(name="pso", bufs=2, space="PSUM"))
        pss = ctx.enter_context(tc.tile_pool(name="pss", bufs=2, space="PSUM"))
        psj = ctx.enter_context(tc.tile_pool(name="psj", bufs=2, space="PSUM"))
        psm = ctx.enter_context(tc.tile_pool(name="psm", bufs=2, space="PSUM"))

        # --- constants -----------------------------------------------------
        # ones first: it gates the PE warm-up burst below
        ones_bf = const.tile([128, 128], CDT, tag="ones", name="ones")
        nc.gpsimd.memset(ones_bf[:], 1.0)

        # PE warm-up: dummy matmuls on the ones tile while the first DMAs are
        # still in flight, so the HAM clock gate is already open (2.4 GHz)
        # when real work arrives.  Sized to bridge until the first wq/x
        # pieces land (~2us) without delaying the first real matmul much.
        warm = psm.tile([128, 512], F32, tag="m", name="warm")
        for _ in range(36):
            nc.tensor.matmul(warm[:, 0:128], ones_bf[:], ones_bf[:],
                             start=True, stop=True, skip_group_check=True)
        wdump = const.tile([128, 128], F32, tag="wdump", name="wdump")
        nc.scalar.copy(wdump[:], warm[:, 0:128])

        # Diagonal causal mask: key row x attends query col y iff y - x >= 0.
        trimask = const.tile([128, 128], CDT, tag="trimask", name="trimask")
        nc.gpsimd.memset(trimask[:], 1.0)
        nc.gpsimd.affine_select(
            out=trimask[:], in_=trimask[:], compare_op=mybir.AluOpType.is_ge,
            fill=0.0, base=0, pattern=[[1, 128]], channel_multiplier=-1)
        ebias = const.tile([128, 1], F32, tag="ebias", name="ebias")
        nc.gpsimd.memset(ebias[:], EXP_BIAS)

        # --- weights: strict bandwidth-priority order on ONE queue.  The
        # first QK unit consumes (wq piece k, xt piece k) pairs in k order;
        # wk is needed one unit later, wv after the first V group, wo only
        # at out-projection.  A parallel queue would steal HBM bandwidth
        # from the critical first pieces, so everything goes on sync in
        # demand order. --------------------------------------------------
        wq_s = wpool.tile([128, ndch * ML], CDT, tag="wq", name="wq_s")
        wk_s = wpool.tile([128, ndch * ML], CDT, tag="wk", name="wk")
        xt0 = xpool.tile([128, ndch * 512], CDT, tag="xt", name="xt")
        w4 = ndch // 4
        # one DMA queue sustains only ~200 GB/s with ~2.5us issue->data
        # latency: split each piece group across the sync and scalar rings,
        # and make the FIRST piece a single k-chunk (wq k0 on sync || x k0
        # on scalar, ~0.19 MB) so the first matmul starts ~2us earlier
        qk_pieces = [(0, 1), (1, 4), (4, 8), (8, 12), (12, 16)]
        for a, bnd in qk_pieces:
            nc.sync.dma_start(
                wq_s[:, a * ML:bnd * ML],
                wqT_e[:, a * ML:bnd * ML])
            if bnd - a == 1:
                nc.scalar.dma_start(
                    xt0[:, a * 512:bnd * 512],
                    xT_e[0, :, a * 512:bnd * 512])
            else:
                nc.sync.dma_start(
                    xt0[:, a * 512:a * 512 + 512],
                    xT_e[0, :, a * 512:a * 512 + 512])
                nc.scalar.dma_start(
                    xt0[:, a * 512 + 512:bnd * 512],
                    xT_e[0, :, a * 512 + 512:bnd * 512])
        for piece in range(4):
            k0 = piece * w4
            nc.sync.dma_start(
                wk_s[:, k0 * ML:(k0 + w4) * ML],
                wkT_e[:, k0 * ML:(k0 + w4) * ML])
        wv_s = wpool.tile([128, ndch * ML], CDT, tag="wv", name="wv")
        nc.sync.dma_start(wv_s[:], wvT_e[:, :])
        wo_s = []
        for h in range(HPC):
            t = wpool.tile([128, d], CDT, tag=f"wo{h}", name=f"wo{h}")
            nc.sync.dma_start(t[:], woT_e[h * 128:(h + 1) * 128, :])
            wo_s.append(t)

        # --- persistent per-batch activations ------------------------------
        QT = [[qkv.tile([128, s], CDT, tag=f"qt{bb}{h}", name=f"qt{bb}{h}") for h in range(HPC)]
              for bb in range(b)]
        KT = [[qkv.tile([128, s], CDT, tag=f"kt{bb}{h}", name=f"kt{bb}{h}") for h in range(HPC)]
              for bb in range(b)]
        V = [qkv.tile([128, nblk * ML], CDT, tag=f"v{bb}", name=f"v{bb}") for bb in range(b)]
        YT = [[qkv.tile([128, s], CDT, tag=f"yt{bb}{h}", name=f"yt{bb}{h}") for h in range(HPC)]
              for bb in range(b)]

        xt_tiles = {(0, 0): xt0}

        def load_xt(bb):
            # emit the x-chunk DMAs early in the sync stream so they are
            # never head-of-line blocked behind output DMAs that wait on
            # attention results
            for tt in range(ttch):
                if (bb, tt) in xt_tiles:
                    continue
                c = (bb * s + tt * 512) // 512
                t = xpool.tile([128, ndch * 512], CDT, tag="xt", name="xt")
                nc.sync.dma_start(t[:], xT_e[c, :, :])
                xt_tiles[(bb, tt)] = t

        def qk_group(bb, tt, m2, wsb, dst):
            xt = xt_tiles[(bb, tt)]
            pst = psj.tile([128, 512], F32, tag="j", name="pst")
            for k in range(ndch):
                nc.tensor.matmul(
                    pst[:],
                    wsb[:, k * ML + m2 * 128: k * ML + m2 * 128 + 128],
                    xt[:, k * 512:(k + 1) * 512],
                    start=(k == 0), stop=(k == ndch - 1))
            # alternate the psum->sbuf cast between DVE and ACT so woven
            # projection groups don't pile onto the attention-loaded DVE
            if m2 % 2 == 0:
                nc.vector.tensor_copy(
                    dst[bb][m2][:, tt * 512:(tt + 1) * 512], pst[:])
            else:
                nc.scalar.copy(
                    dst[bb][m2][:, tt * 512:(tt + 1) * 512], pst[:])

        def qk_pair(bb, tt, wsb, dst, pool, ptag):
            # both m2 groups interleaved piece-by-piece: during startup the
            # weight/x pieces stream in at ~2.1us each while one group
            # consumes a piece in ~0.9us -- interleaving doubles the PE work
            # per piece so the DMA stream stays ahead.  The wk pair gets the
            # (still idle) score pool so it never waits on the wq pair's
            # psum casts.
            xt = xt_tiles[(bb, tt)]
            psts = [pool.tile([128, 512], F32, tag=ptag, name=f"pst{m2}")
                    for m2 in range(HPC)]
            for a, bnd in qk_pieces:
                for m2 in range(HPC):
                    for k in range(a, bnd):
                        nc.tensor.matmul(
                            psts[m2][:],
                            wsb[:, k * ML + m2 * 128: k * ML + m2 * 128 + 128],
                            xt[:, k * 512:(k + 1) * 512],
                            start=(k == 0), stop=(k == ndch - 1))
            for m2 in range(HPC):
                if m2 % 2 == 0:
                    nc.vector.tensor_copy(
                        dst[bb][m2][:, tt * 512:(tt + 1) * 512], psts[m2][:])
                else:
                    nc.scalar.copy(
                        dst[bb][m2][:, tt * 512:(tt + 1) * 512], psts[m2][:])

        def v_group(bb, tt, v4):
            xt = xt_tiles[(bb, tt)]
            pst = psj.tile([128, 512], F32, tag="j", name="pst")
            for k in range(ndch):
                nc.tensor.matmul(
                    pst[:, 0:ML],
                    xt[:, k * 512 + v4 * 128: k * 512 + v4 * 128 + 128],
                    wv_s[:, k * ML:(k + 1) * ML],
                    start=(k == 0), stop=(k == ndch - 1))
            blk = tt * 4 + v4
            nc.scalar.copy(V[bb][:, blk * ML:(blk + 1) * ML], pst[:, 0:ML])

        def proj_units(bb):
            # phase 1 as a list of independently emittable 16-matmul units so
            # the next batch's projection can be woven between attention
            # blocks of the current one (the per-engine schedule is
            # priority-ordered: stalls can only be filled by work that is
            # EMITTED inside the stall region)
            units = []
            for tt in range(ttch):
                # all wq units before wk units: matches the DMA priority
                # order at startup (the PE queue is in-order, so a unit
                # waiting on late data blocks every later-emitted unit).
                # The very first chunk uses the piece-interleaved pair form
                # to ride the incoming DMA stream without stalls.
                if bb == 0 and tt == 0:
                    units.append(lambda: qk_pair(0, 0, wq_s, QT, psj, "j"))
                    units.append(lambda: qk_pair(0, 0, wk_s, KT, pss, "s"))
                else:
                    for wsb, dst in ((wq_s, QT), (wk_s, KT)):
                        for m2 in range(HPC):
                            units.append(
                                lambda bb=bb, tt=tt, m2=m2, wsb=wsb, dst=dst:
                                qk_group(bb, tt, m2, wsb, dst))
                for v4 in range(4):
                    units.append(
                        lambda bb=bb, tt=tt, v4=v4: v_group(bb, tt, v4))
            return units

        def outproj_block(bb, n):
            # --- phase 3: partial out-projection for one 128-token block ---
            # fine-grained staging (one tile + DMA per 512-wide e chunk) so
            # the 32 MB output stream drains smoothly in the background
            t0 = n * 128
            # full-width (128, 2048) fp16 staging: one DMA per token block
            # with 4 KiB contiguous rows (2 KiB rows measured at only ~180
            # GB/s -- half rate); psum copies alternate ACT/DVE
            ost = opool.tile([128, d], CDT, tag="ost", name="ost")
            for ec in range(nech):
                # alternate psm/psj: 4 evacuation buffers keep the matmul
                # pairs (430ns/ec) ahead of the ~690ns psum->sbuf copies in
                # pure-outproj stretches (psj only carries the brief
                # denominator broadcast in the last batch's window)
                if ec % 2 == 0:
                    ps_p = psm.tile([128, 512], F32, tag="m", name="ps_p")
                else:
                    ps_p = psj.tile([128, 512], F32, tag="j", name="ps_p")
                for h in range(HPC):
                    nc.tensor.matmul(
                        ps_p[:],
                        YT[bb][h][:, t0:t0 + 128],
                        wo_s[h][:, ec * 512:(ec + 1) * 512],
                        start=(h == 0), stop=(h == HPC - 1))
                if ec % 2 == 0:
                    nc.scalar.copy(ost[:, ec * 512:(ec + 1) * 512], ps_p[:])
                else:
                    nc.vector.tensor_copy(
                        ost[:, ec * 512:(ec + 1) * 512], ps_p[:])
            nc.sync.dma_start(
                out_e[bb * s + t0: bb * s + t0 + 128, :], ost[:])

        def drain(fillers, k):
            for _ in range(min(k, len(fillers))):
                fillers.pop(0)()

        def attn_block(bb, qj, fillers=None, defer_to=None):
            # --- phase 2: causal attention for one 512-query block ---------
            for h in range(HPC):
                nki = 4 * qj + 4
                q0 = qj * 512
                ps_o = pso.tile([128, 512], F32, tag="o", name="ps_o")
                # single fp16 DVE accumulator chain: at ~370ns per add it
                # keeps up with et production (~520ns/tile), and the DVE is
                # in-order so a second chain would buy no latency -- while
                # one chain needs only ONE broadcast matmul per head-block
                acc = dpool.tile([128, 512], CDT, tag="acc", name="acc")
                for ki in range(nki):
                    # diagonal key chunks only see queries >= their own
                    # position: narrow to columns [c0:512)
                    g = ki - 4 * qj
                    c0 = 128 * g if g > 0 else 0
                    ps_s = pss.tile([128, 512], F32, tag="s", name="ps_s")
                    nc.tensor.matmul(
                        ps_s[:, c0:512],
                        KT[bb][h][:, ki * 128:(ki + 1) * 128],
                        QT[bb][h][:, q0 + c0:q0 + 512],
                        start=True, stop=True)
                    et = epool.tile([128, 512], CDT, tag="et", name="et")
                    nc.scalar.activation(
                        et[:, c0:512], ps_s[:, c0:512],
                        mybir.ActivationFunctionType.Exp, scale=SCALE,
                        bias=ebias[:, 0:1])
                    if g >= 0:
                        nc.vector.tensor_mul(
                            et[:, c0:c0 + 128], et[:, c0:c0 + 128],
                            trimask[:])
                    if ki == 0:
                        nc.vector.tensor_copy(acc[:, c0:512], et[:, c0:512])
                    else:
                        nc.vector.tensor_add(acc[:, c0:512], acc[:, c0:512],
                                             et[:, c0:512])
                    nc.tensor.matmul(
                        ps_o[:, c0:512],
                        V[bb][:, ki * ML + h * 128: ki * ML + h * 128 + 128],
                        et[:, c0:512],
                        start=(ki == 0), stop=(ki == nki - 1),
                        skip_group_check=True)
                    if fillers and ki % 3 == 2:
                        drain(fillers, 1)
                rbc = dpool.tile([128, 512], F32, tag="rbc", name="rbc")
                # one accumulating ones(128,128) matmul sums the 128
                # partitions of acc and broadcasts the denominator across
                # all 128 output partitions.  In the last batch the
                # projection PSUM pool (psj) is idle: use it there to keep
                # the broadcast off the outproj-churned psm pool.
                bpool = psj if bb + 1 == b else psm
                ps_r = bpool.tile([128, 512], F32, tag="j" if bb + 1 == b
                                  else "m", name="ps_r")
                nc.tensor.matmul(ps_r[:], ones_bf[:], acc[:],
                                 start=True, stop=True)
                nc.vector.reciprocal_approx_fast(out=rbc[:], in_=ps_r[:])
                nc.vector.tensor_mul(YT[bb][h][:, q0:q0 + 512],
                                     ps_o[:], rbc[:])
            for n in range(4 * qj, 4 * qj + 4):
                if defer_to is not None:
                    defer_to.append(lambda bb=bb, n=n: outproj_block(bb, n))
                else:
                    if fillers:
                        drain(fillers, 1)
                    outproj_block(bb, n)

        # per-batch stagger: the next batch's projection units are emitted
        # INSIDE the current batch's attention blocks (the per-engine
        # schedule is priority-ordered, so softmax-latency stalls can only
        # be filled by work emitted within them).  qj descending: longest
        # attention blocks first.
        deferred = []
        for bb in range(b):
            load_xt(bb)
            if bb == 0:
                for u in proj_units(0):
                    u()
            # fillers for this batch's attention window: the previous
            # batch's deferred out-projection plus the next batch's
            # projection units
            fillers = list(deferred)
            deferred = []
            if bb + 1 < b:
                load_xt(bb + 1)
                fillers += proj_units(bb + 1)
                defer_to = deferred   # push own out-proj into next window
            else:
                defer_to = None
            if bb + 1 < b:
                order = list(range(nqj - 1, -1, -1))
            else:
                # last batch: qj0 first (ready after the first projection
                # chunk, warms the softmax chain early), then longest-first
                order = [0] + list(range(nqj - 1, 0, -1))
            for qj in order:
                attn_block(bb, qj, fillers, defer_to)
            for u in fillers:
                u()

    nc.compile()
    return nc


_NC_CACHE = {}


def _get_nc():
    if "nc" not in _NC_CACHE:
        _NC_CACHE["nc"] = build_nc()
    return _NC_CACHE["nc"]


def _pretile(wT):
    """[d, m] -> [128, (d//128)*m] with per-partition-contiguous k-chunks:
    out[p, k*m + j] = wT[k*128 + p, j].  Makes every DMA a long unit-stride
    run per partition (fast descriptor gen + full HBM burst efficiency)."""
    d, m = wT.shape
    return np.ascontiguousarray(
        wT.reshape(d // 128, 128, m).transpose(1, 0, 2).reshape(128, -1))


def shard_inputs(x, wq, wk, wv, wo):
    """Host-side sharding: 2 heads (256 out dims) per core; fp16 cast."""
    ntok = x.shape[0] * x.shape[1]
    xf = np.asarray(x, dtype=np.float32).reshape(ntok, D)
    # x pre-tiled per 512-token chunk: xT3[c, p, k*512 + t] = x[c*512+t, k*128+p]
    nch = ntok // 512
    xT3 = np.ascontiguousarray(
        xf.reshape(nch, 512, D // 128, 128).transpose(0, 3, 2, 1)
        .reshape(nch, 128, -1)).astype(NPCDT)
    in_maps = []
    for c in range(N_CORES):
        sl = slice(c * ML, (c + 1) * ML)
        in_maps.append({
            "xT": xT3,
            "wqT": _pretile(np.asarray(wq)[sl].T).astype(NPCDT),
            "wkT": _pretile(np.asarray(wk)[sl].T).astype(NPCDT),
            "wvT": _pretile(np.asarray(wv)[sl].T).astype(NPCDT),
            "woT": np.ascontiguousarray(np.asarray(wo)[:, sl].T).astype(NPCDT),
        })
    return in_maps


def run(inputs, trace=False, trace_cores=None):
    nc = _get_nc()
    in_maps = shard_inputs(inputs["x"], inputs["wq"], inputs["wk"],
                           inputs["wv"], inputs["wo"])
    res = run_bass_kernel_spmd(nc, in_maps, core_ids=list(range(N_CORES)),
                               trace=trace, trace_cores=trace_cores)
    out = res.results[0]["out"].astype(np.float32)
    for c in range(1, N_CORES):
        out = out + res.results[c]["out"]
    return out.reshape(B, S, D), res


def kernel(**inputs) -> np.ndarray:
    out, _ = run(inputs, trace=False)
    return out

